# revision 23
# baseline (speedup 1.0000x reference)
"""Causal self-attention (B=1, T=4096, C=768, H=12, D=64) on 8 TRN2 NeuronCores.

Sharding: 8 cores = 4 head-groups (3 heads each) x 2 sequence-groups.
Core c: heads [3*hg, 3*hg+2] where hg=c//2; handles q-chunks of 256 rows,
global chunk g = 2*j + s (s=c%2, j=0..7) -- interleaving balances the causal
triangle so every core runs an identical instruction stream (SPMD), with the
boundary masks supplied as per-core data.

v4 design notes:
- S^T via diag-packed K^T (K=128 contraction; 64-partition matmuls measure
  ~2.5x slower, so the zero-diag trick is the fast path).  kt[h] is [128,T]:
  per 128-col block, top rows hold the even 64 columns' K^T, bottom rows the
  odd 64; Q^T is duplicated vertically (weight-side duplication).  The diag
  layout is written directly from phase 1 with strided bias-adds; the zero
  halves are strided-memset once.
- Flash-style: softmax denominator comes free as the ones-column row of the
  PV product; no running max (logits O(1) off-diagonal, diag ~ |q|^2/8).
- Off-diagonal (non-boundary) groups: exp bias -2 -> P fits fp8e4; PV runs
  fp8 DoubleRow matmuls contracting 2 k-blocks per call.
- Boundary group holds the diagonal (logit to ~13): exp bias -6 in fp16 and
  the fp16 V copy is pre-scaled by e^4, so both paths contribute P*e^-2*V.
- V^T computed directly (x^T-chunk as stationary operand) -> no PE
  transposes; V bias via a K=1 ones matmul.  vp16 (fp16, x e^4) written by
  DVE; vp8 (= vp16 * e^-4) derived on the idle GpSimd engine.
- Phase-1 K/V quarters are interleaved between attention slot pairs so the
  PE never idles long enough for HAM to re-throttle the clock.
- Projection accumulates 3 heads in PSUM; fp16 staging tile; host adds
  b_proj and the 4 head-group partials in f64.
"""
import numpy as np

T, C, H, D = 4096, 768, 12, 64
NH = 3          # heads per core
QC = 256        # q rows per slot
P = 128
FB = 3.0        # fp8-path exp shift: exp(z-FB) must stay under fp8e4 max
BB = 6.5        # boundary-group exp shift (diag logit up to ~17 in fp16)
EB = BB - FB    # V16 pre-scale e^EB makes both paths contribute P*e^-FB*V

_nc_cache = {}


def split_multi_waits(nc):
    """Walrus here accepts only one sync wait per instruction: hoist extras
    onto standalone InstEventSemaphore instructions on the same engine."""
    import concourse.mybir as mybir
    n_split = 0
    for f in nc.m.functions:
        for bb in f.blocks:
            new_insts = []
            for inst in bb.instructions:
                si = inst.sync_info
                if si is not None and len(si.on_wait) > 1:
                    for w in si.on_wait[:-1]:
                        nop = mybir.InstEventSemaphore(
                            name=nc.get_next_instruction_name(), ins=[], outs=[])
                        nop.engine = inst.engine
                        nop.sync_info = mybir.SyncInfo(on_wait=[w], on_update=[])
                        nc.register_instruction(nop)
                        new_insts.append(nop)
                        n_split += 1
                    si.on_wait = si.on_wait[-1:]
                new_insts.append(inst)
            bb.instructions[:] = new_insts
    return n_split


def build_nc(Tloc=T):
    import concourse.bass as bass
    import concourse.mybir as mybir
    import concourse.tile as tile
    from contextlib import ExitStack

    f32r = mybir.dt.float32r
    f32 = mybir.dt.float32
    f16 = mybir.dt.float16
    f8 = mybir.dt.float8e4
    EXP = mybir.ActivationFunctionType.Exp
    ADD = mybir.AluOpType.add
    MUL = mybir.AluOpType.mult
    DR = mybir.MatmulPerfMode.DoubleRow

    nslot = Tloc // (2 * QC)     # q-chunks per core (8)
    nkb = Tloc // P              # k 128-blocks (32)
    nkp = nkb // 2               # k 256-block pairs (16)
    tq = nslot * QC              # q rows per core (2048)
    QT4 = Tloc // 4              # phase-1 quarter width (1024)

    nc = bass.Bass(trn_type="TRN2")
    xt16 = nc.dram_tensor("xt16", [C, Tloc], f16, kind="ExternalInput")
    xtq16 = nc.dram_tensor("xtq16", [C, tq], f16, kind="ExternalInput")
    wk01x = nc.dram_tensor("wk01", [C, P], f16, kind="ExternalInput")
    wk2x = nc.dram_tensor("wk2", [C, D], f16, kind="ExternalInput")
    wv3x = nc.dram_tensor("wv3", [C, NH * D], f16, kind="ExternalInput")
    # per head, W_q columns duplicated [Qh|Qh] for the diag-packed S
    wqdx = nc.dram_tensor("wqd", [C, NH * P], f16, kind="ExternalInput")
    wpjx = nc.dram_tensor("wpj", [NH, D, C], f16, kind="ExternalInput")
    # bias cols: 0 K0|K1, 1 K2|-, 2 Q0|Q0, 3 Q1|Q1, 4 Q2|Q2
    biasx = nc.dram_tensor("bias", [P, 5], f32, kind="ExternalInput")
    bvx = nc.dram_tensor("bv16", [1, NH * D], f16, kind="ExternalInput")
    maskx = nc.dram_tensor("mask", [nslot, P, 1024], f16, kind="ExternalInput")
    out = nc.dram_tensor("out", [tq, C], f16, kind="ExternalOutput")

    with tile.TileContext(nc) as tc, ExitStack() as ctx:
        singles = ctx.enter_context(tc.tile_pool(name="singles", bufs=1))
        xthp = ctx.enter_context(tc.tile_pool(name="xth", bufs=2))
        xqp = ctx.enter_context(tc.tile_pool(name="xq", bufs=2))
        qtp = ctx.enter_context(tc.tile_pool(name="qt", bufs=2))
        mtp = ctx.enter_context(tc.tile_pool(name="mt", bufs=2))
        ptp = ctx.enter_context(tc.tile_pool(name="pt", bufs=3))
        rbp = ctx.enter_context(tc.tile_pool(name="rb", bufs=2))
        ytp = ctx.enter_context(tc.tile_pool(name="yt", bufs=2))
        ostp = ctx.enter_context(tc.tile_pool(name="ost", bufs=2))
        psg = ctx.enter_context(tc.tile_pool(name="psg", bufs=2, space="PSUM"))
        psy = ctx.enter_context(tc.tile_pool(name="psy", bufs=2, space="PSUM"))
        psm = ctx.enter_context(tc.tile_pool(name="psm", bufs=2, space="PSUM"))

        ones_f = singles.tile([1, 64], f32)
        nc.vector.memset(ones_f, 1.0)
        ones64 = singles.tile([1, 64], f32r)
        nc.vector.tensor_copy(ones64, ones_f)
        ones1h = singles.tile([1, P], f16)
        nc.vector.memset(ones1h, 1.0)
        # exp biases: softmax is shift-invariant.  -2 keeps off-diagonal
        # exp() inside fp8e4 range; the boundary group (holds the diagonal,
        # logit ~ |q|^2/8 up to ~13) uses -2-EB with V16 pre-scaled by e^EB.
        en2 = singles.tile([P, 1], f32)
        nc.vector.memset(en2, -FB)
        en6 = singles.tile([P, 1], f32)
        nc.vector.memset(en6, -BB)

        wk01_t = singles.tile([P, 6, P], f16)
        wk2_t = singles.tile([P, 6, D], f16)
        wv3_t = singles.tile([P, 6, NH * D], f16)
        wqd_t = singles.tile([P, 6, NH * P], f16)
        for c in range(6):
            nc.sync.dma_start(wk01_t[:, c], wk01x[P * c:P * c + P, :])
            nc.sync.dma_start(wk2_t[:, c], wk2x[P * c:P * c + P, :])
            nc.sync.dma_start(wv3_t[:, c], wv3x[P * c:P * c + P, :])
            nc.sync.dma_start(wqd_t[:, c], wqdx[P * c:P * c + P, :])
        wpj_t = []
        for h in range(NH):
            w1 = singles.tile([D, C], f16, tag=f"wpj{h}", name=f"wpj{h}")
            nc.sync.dma_start(w1, wpjx[h])
            wpj_t.append(w1)
        b_t = singles.tile([P, 5], f32)
        nc.sync.dma_start(b_t, biasx[:, :])
        bv_t = singles.tile([1, NH * D], f16)
        nc.sync.dma_start(bv_t, bvx[:, :])

        # kt[h]: [128, T] diag-packed K^T.  Zero halves memset once (strided).
        kt_t = [singles.tile([P, Tloc], f16, tag=f"kt{h}", name=f"kt{h}")
                for h in range(NH)]
        for h in range(NH):
            v = kt_t[h].rearrange("p (b t) -> p b t", t=P)
            nc.vector.memset(v[0:64, :, 64:P], 0.0)
            nc.vector.memset(v[64:P, :, 0:64], 0.0)
        # vp8[h]: [128 k-part, pair, slab(2), 80]; col 64 = ones (denom).
        # vp16[h]: fp16 V^T copy scaled by e^EB for the boundary group.
        vp_t = [singles.tile([P, nkp, 2, 80], f8, tag=f"vp{h}", name=f"vp{h}")
                for h in range(NH)]
        vp16_t = [singles.tile([P, nkb, 65], f16, tag=f"vq{h}", name=f"vq{h}")
                  for h in range(NH)]
        for h in range(NH):
            nc.vector.memset(vp_t[h][:, :, :, 64:65], 1.0)
            nc.vector.memset(vp16_t[h][:, :, 64:65], float(np.exp(EB)))

        # ---- phase 1: K^T rows + V^T blocks for one column-quarter ----
        def p1_load(q):
            xh = xthp.tile([P, 6, QT4], f16, tag="xh", name=f"xh{q}")
            for c in range(6):
                nc.sync.dma_start(
                    xh[:, c], xt16[P * c:P * c + P, QT4 * q:QT4 * (q + 1)])
            return xh

        def kt_write(h, rows, g0, ps):
            """psum rows ([64] K^T dims x 512 t-cols) -> diag layout of kt[h]:
            even 64-col halves to top rows, odd halves to bottom rows."""
            pv = ps[rows].rearrange("p (b t) -> p b t", t=P)
            kv = kt_t[h][:, g0:g0 + 512].rearrange("p (b t) -> p b t", t=P)
            bsl = b_t[rows, 0:1] if h < 2 else b_t[0:64, 1:2]
            nc.vector.tensor_scalar(kv[0:64, :, 0:64], pv[:, :, 0:64],
                                    bsl, None, ADD)
            nc.vector.tensor_scalar(kv[64:P, :, 64:P], pv[:, :, 64:P],
                                    bsl, None, ADD)

        def p1_compute(q, xh):
            for n0 in (0, 512):
                g0 = QT4 * q + n0
                # K0|K1 packed on output partitions
                ps = psm.tile([P, 512], f32, tag="psm", name="psK")
                for c in range(6):
                    nc.tensor.matmul(ps, wk01_t[:, c], xh[:, c, n0:n0 + 512],
                                     start=(c == 0), stop=(c == 5))
                kt_write(0, slice(0, 64), g0, ps)
                kt_write(1, slice(64, P), g0, ps)
                # K2
                ps2 = psm.tile([P, 512], f32, tag="psm", name="psK2")
                for c in range(6):
                    nc.tensor.matmul(ps2[0:64], wk2_t[:, c], xh[:, c, n0:n0 + 512],
                                     start=(c == 0), stop=(c == 5))
                kt_write(2, slice(0, 64), g0, ps2)
                # V^T: x^T block as stationary operand -> [128 t, 192]
                for bi in range(4):
                    t0 = n0 + P * bi
                    blk = g0 // P + bi
                    vps = psm.tile([P, 512], f32, tag="psm", name="psV")
                    for c in range(6):
                        nc.tensor.matmul(vps[:, :NH * D], xh[:, c, t0:t0 + P],
                                         wv3_t[:, c], start=(c == 0), stop=False)
                    nc.tensor.matmul(vps[:, :NH * D], ones1h, bv_t,
                                     start=False, stop=True)
                    for h in range(NH):
                        nc.vector.tensor_scalar(
                            vp16_t[h][:, blk, 0:64], vps[:, D * h:D * h + D],
                            float(np.exp(EB)), None, MUL)
                        nc.gpsimd.tensor_scalar(
                            vp_t[h][:, blk // 2, blk % 2, 0:64],
                            vp16_t[h][:, blk, 0:64],
                            float(np.exp(-EB)), None, MUL)

        # ---- Q^T (duplicated rows) for a slot pair (2 x 256 q rows) ----
        def q_proj(jp):
            xq = xqp.tile([P, 6, 512], f16, tag="xq", name="xq")
            for c in range(6):
                nc.sync.dma_start(
                    xq[:, c], xtq16[P * c:P * c + P, 512 * jp:512 * (jp + 1)])
            qts = []
            for h in range(NH):
                psq = psm.tile([P, 512], f32, tag="psm", name="psq")
                for c in range(6):
                    nc.tensor.matmul(psq, wqd_t[:, c, P * h:P * (h + 1)],
                                     xq[:, c], start=(c == 0), stop=(c == 5))
                qh = qtp.tile([P, 512], f16, tag=f"qt{h}", name=f"qt{h}")
                nc.vector.tensor_scalar(qh, psq, b_t[:, 2 + h:3 + h], None, ADD)
                qts.append(qh)
            return qts

        # ---- attention + projection for one slot ----
        def do_slot(j, qts):
            qoff = 256 * (j % 2)
            mt = mtp.tile([P, 1024], f16, tag="mt", name="mt")
            nc.sync.dma_start(mt, maskx[j])

            yt_t = []
            for h in range(NH):
                yacc = psy.tile([80, QC], f32, tag="yacc", name="yacc")
                qsl = qts[h][:, qoff:qoff + QC]

                def s_group(g):
                    sg = psg.tile([P, 1024], f32, tag="sg", name="sg")
                    for i in range(4):
                        kb = 4 * g + i
                        nc.tensor.matmul(sg[:, QC * i:QC * (i + 1)],
                                         kt_t[h][:, P * kb:P * (kb + 1)],
                                         qsl, start=True, stop=True)
                    return sg

                # software pipeline: issue S(g+1) before PV(g) so the PE has
                # work while ACT runs exp(g)
                sg_cur = s_group(0)
                for g in range(j + 1):
                    sg_next = s_group(g + 1) if g < j else None
                    if g < j:
                        # off-diagonal group: fp8 P + DoubleRow PV
                        pt = ptp.tile([P, 1024], f8, tag="pt", name="pt")
                        nc.scalar.activation(pt, sg_cur, EXP,
                                             bias=en2, scale=0.125)
                        for i2 in range(2):
                            nc.tensor.matmul(
                                yacc,
                                vp_t[h][:, 2 * g + i2],
                                pt[:, 512 * i2:512 * (i2 + 1)].rearrange(
                                    "p (s q) -> p s q", s=2),
                                start=(g == 0 and i2 == 0), stop=False,
                                perf_mode=DR)
                    else:
                        # boundary group holds the diagonal (logit ~ |q|^2/8,
                        # overflows fp8 and, at -2, even fp16): fp16 P with
                        # bias -2-EB against V16 pre-scaled by e^EB, masked
                        pt = ptp.tile([P, 1024], f16, tag="pt16", name="pt16")
                        nc.scalar.activation(pt, sg_cur, EXP,
                                             bias=en6, scale=0.125)
                        nc.gpsimd.tensor_mul(pt, pt, mt)
                        for i in range(4):
                            kb = 4 * g + i
                            nc.tensor.matmul(yacc[0:65], vp16_t[h][:, kb],
                                             pt[:, QC * i:QC * (i + 1)],
                                             start=(g == 0 and i == 0),
                                             stop=(i == 3))
                    sg_cur = sg_next
                # y^T = yacc[0:64] / den; den = ones-row 64 of yacc, replicated
                # to 64 partitions via a K=1 ones matmul, then reciprocal
                den = rbp.tile([1, QC], f32r, tag="den", name="den")
                nc.vector.tensor_copy(den, yacc[64:65])
                bc = psm.tile([P, 512], f32, tag="psm", name="bc")
                nc.tensor.matmul(bc[0:64, 0:QC], ones64, den,
                                 start=True, stop=True)
                rb = rbp.tile([D, QC], f32, tag="rb", name="rb")
                nc.vector.reciprocal(rb, bc[0:64, 0:QC])
                yt = ytp.tile([D, QC], f16, tag=f"yt{h}", name=f"yt{h}")
                nc.vector.tensor_mul(yt, yacc[0:64], rb)
                yt_t.append(yt)

            ost = ostp.tile([P, 2, C], f16, tag="ost", name="ost")
            for qb in range(2):
                for (n0, nw) in [(0, 512), (512, 256)]:
                    pp = psm.tile([P, 512], f32, tag="psm", name="pp")
                    for h in range(NH):
                        nc.tensor.matmul(pp[:, :nw], yt_t[h][:, P * qb:P * (qb + 1)],
                                         wpj_t[h][:, n0:n0 + nw],
                                         start=(h == 0), stop=(h == NH - 1))
                    nc.vector.tensor_copy(ost[:, qb, n0:n0 + nw], pp[:, :nw])
            for qb in range(2):
                nc.sync.dma_start(
                    out[QC * j + P * qb:QC * j + P * (qb + 1), :], ost[:, qb])

        # ---- main schedule: interleave phase-1 quarters with slot pairs ----
        xh = p1_load(0)
        p1_compute(0, xh)
        for jp in range(nslot // 2):
            if jp + 1 < 4:
                xh = p1_load(jp + 1)
            qts = q_proj(jp)
            do_slot(2 * jp, qts)
            do_slot(2 * jp + 1, qts)
            if jp + 1 < 4:
                p1_compute(jp + 1, xh)

    split_multi_waits(nc)
    return nc


def make_in_maps(x, W_qkv, b_qkv, W_proj, Tloc=T):
    """Shard the full inputs into the 8 per-core input maps."""
    nslot = Tloc // (2 * QC)
    xT = np.ascontiguousarray(x.reshape(Tloc, C).T).astype(np.float32)
    xT16 = xT.astype(np.float16)

    kk = np.arange(P)
    qq = np.arange(QC)
    in_maps = []
    for core in range(8):
        hg, s = core // 2, core % 2
        heads = [3 * hg + i for i in range(NH)]
        wk = [W_qkv[:, C + 64 * h:C + 64 * h + 64] for h in heads]
        wv = [W_qkv[:, 2 * C + 64 * h:2 * C + 64 * h + 64] for h in heads]
        wq = [W_qkv[:, 64 * h:64 * h + 64] for h in heads]
        wk01_c = np.concatenate(wk[0:2], axis=1).astype(np.float16)
        wk2_c = np.ascontiguousarray(wk[2]).astype(np.float16)
        wv3_c = np.concatenate(wv, axis=1).astype(np.float16)
        wqd_c = np.concatenate([np.tile(w, (1, 2)) for w in wq],
                               axis=1).astype(np.float16)
        wpj_c = np.stack([W_proj[64 * h:64 * h + 64, :] for h in heads]
                         ).astype(np.float16)

        bk = [b_qkv[C + 64 * h:C + 64 * h + 64] for h in heads]
        bv = [b_qkv[2 * C + 64 * h:2 * C + 64 * h + 64] for h in heads]
        bq = [b_qkv[64 * h:64 * h + 64] for h in heads]
        bias_c = np.zeros((P, 5), np.float32)
        bias_c[0:64, 0] = bk[0]
        bias_c[64:P, 0] = bk[1]
        bias_c[0:64, 1] = bk[2]
        for hi in range(NH):
            bias_c[0:64, 2 + hi] = bq[hi]
            bias_c[64:P, 2 + hi] = bq[hi]
        bv_c = np.concatenate(bv).reshape(1, NH * 64).astype(np.float16)

        qcols = np.concatenate(
            [np.arange(QC * (2 * j + s), QC * (2 * j + s) + QC)
             for j in range(nslot)])
        xtq_16 = np.ascontiguousarray(xT16[:, qcols])

        mask_c = np.zeros((nslot, P, 1024), np.float32)
        for j in range(nslot):
            q0 = QC * (2 * j + s)
            for i in range(4):
                k0 = P * (4 * j + i)
                mask_c[j, :, QC * i:QC * (i + 1)] = (
                    (k0 + kk[:, None]) <= (q0 + qq[None, :]))

        in_maps.append({
            "xt16": xT16, "xtq16": xtq_16,
            "wk01": wk01_c, "wk2": wk2_c, "wv3": wv3_c,
            "wqd": wqd_c, "wpj": wpj_c,
            "bias": bias_c, "bv16": bv_c,
            "mask": mask_c.astype(np.float16),
        })
    return in_maps


def unshard(results, b_proj, Tloc=T):
    nslot = Tloc // (2 * QC)
    out = np.zeros((Tloc, C), np.float64)
    for core in range(8):
        s = core % 2
        r = np.asarray(results[core]["out"]).astype(np.float64)
        for j in range(nslot):
            g0 = QC * (2 * j + s)
            out[g0:g0 + QC] += r[QC * j:QC * (j + 1)]
    out += b_proj.astype(np.float64)
    return out.astype(np.float32).reshape(1, Tloc, C)


_last_result = {}


def kernel(x, mask, W_qkv, b_qkv, W_proj, b_proj):
    from concourse.bass_utils import run_bass_kernel_spmd
    x = np.asarray(x, np.float32)
    W_qkv = np.asarray(W_qkv, np.float32)
    b_qkv = np.asarray(b_qkv, np.float32)
    W_proj = np.asarray(W_proj, np.float32)
    b_proj = np.asarray(b_proj, np.float32)

    if "nc" not in _nc_cache:
        _nc_cache["nc"] = build_nc(T)
    nc = _nc_cache["nc"]
    in_maps = make_in_maps(x, W_qkv, b_qkv, W_proj, T)
    import os
    kwargs = {}
    if os.environ.get("BASS_KERNEL_TRACE"):
        kwargs = dict(trace=True, trace_cores=list(range(8)))
    res = run_bass_kernel_spmd(nc, in_maps, core_ids=list(range(8)), **kwargs)
    _last_result["res"] = res
    return unshard([r for r in res.results], b_proj, T)


# revision 28
# speedup vs baseline: 1.3324x; 1.3324x over previous
"""Causal self-attention (B=1, T=4096, C=768, H=12, D=64) on 8 TRN2 NeuronCores.

Sharding: 8 cores = 4 head-groups (3 heads each) x 2 sequence-groups.
Core c: heads [3*hg, 3*hg+2] where hg=c//2; handles q-chunks of 256 rows,
global chunk g = 2*j + s (s=c%2, j=0..7) -- interleaving balances the causal
triangle so every core runs an identical instruction stream (SPMD), with the
boundary masks supplied as per-core data.

v4 design notes:
- S^T via diag-packed K^T (K=128 contraction; 64-partition matmuls measure
  ~2.5x slower, so the zero-diag trick is the fast path).  kt[h] is [128,T]:
  per 128-col block, top rows hold the even 64 columns' K^T, bottom rows the
  odd 64; Q^T is duplicated vertically (weight-side duplication).  The diag
  layout is written directly from phase 1 with strided bias-adds; the zero
  halves are strided-memset once.
- Flash-style: softmax denominator comes free as the ones-column row of the
  PV product; no running max (logits O(1) off-diagonal, diag ~ |q|^2/8).
- Off-diagonal (non-boundary) groups: exp bias -2 -> P fits fp8e4; PV runs
  fp8 DoubleRow matmuls contracting 2 k-blocks per call.
- Boundary group holds the diagonal (logit to ~13): exp bias -6 in fp16 and
  the fp16 V copy is pre-scaled by e^4, so both paths contribute P*e^-2*V.
- V^T computed directly (x^T-chunk as stationary operand) -> no PE
  transposes; V bias via a K=1 ones matmul.  vp16 (fp16, x e^4) written by
  DVE; vp8 (= vp16 * e^-4) derived on the idle GpSimd engine.
- Phase-1 K/V quarters are interleaved between attention slot pairs so the
  PE never idles long enough for HAM to re-throttle the clock.
- Projection accumulates 3 heads in PSUM; fp16 staging tile; host adds
  b_proj and the 4 head-group partials in f64.
"""
import numpy as np

T, C, H, D = 4096, 768, 12, 64
NH = 3          # heads per core
QC = 256        # q rows per slot
P = 128
FB = 3.0        # fp8-path exp shift: exp(z-FB) must stay under fp8e4 max
BB = 6.5        # boundary-group exp shift (diag logit up to ~17 in fp16)
EB = BB - FB    # V16 pre-scale e^EB makes both paths contribute P*e^-FB*V

_nc_cache = {}


def split_multi_waits(nc):
    """Walrus here accepts only one sync wait per instruction: hoist extras
    onto standalone InstEventSemaphore instructions on the same engine."""
    import concourse.mybir as mybir
    n_split = 0
    for f in nc.m.functions:
        for bb in f.blocks:
            new_insts = []
            for inst in bb.instructions:
                si = inst.sync_info
                if si is not None and len(si.on_wait) > 1:
                    for w in si.on_wait[:-1]:
                        nop = mybir.InstEventSemaphore(
                            name=nc.get_next_instruction_name(), ins=[], outs=[])
                        nop.engine = inst.engine
                        nop.sync_info = mybir.SyncInfo(on_wait=[w], on_update=[])
                        nc.register_instruction(nop)
                        new_insts.append(nop)
                        n_split += 1
                    si.on_wait = si.on_wait[-1:]
                new_insts.append(inst)
            bb.instructions[:] = new_insts
    return n_split


def build_nc(Tloc=T):
    import concourse.bass as bass
    import concourse.mybir as mybir
    import concourse.tile as tile
    from contextlib import ExitStack

    f32r = mybir.dt.float32r
    f32 = mybir.dt.float32
    f16 = mybir.dt.float16
    f8 = mybir.dt.float8e4
    EXP = mybir.ActivationFunctionType.Exp
    LOG = mybir.ActivationFunctionType.Log
    ADD = mybir.AluOpType.add
    MUL = mybir.AluOpType.mult
    DR = mybir.MatmulPerfMode.DoubleRow

    nslot = Tloc // (2 * QC)     # q-chunks per core (8)
    nkb = Tloc // P              # k 128-blocks (32)
    nkp = nkb // 2               # k 256-block pairs (16)
    tq = nslot * QC              # q rows per core (2048)
    QT4 = Tloc // 4              # phase-1 quarter width (1024)

    nc = bass.Bass(trn_type="TRN2")
    xt16 = nc.dram_tensor("xt16", [C, Tloc], f16, kind="ExternalInput")
    xtq16 = nc.dram_tensor("xtq16", [C, tq], f16, kind="ExternalInput")
    wk01x = nc.dram_tensor("wk01", [C, P], f16, kind="ExternalInput")
    wk2x = nc.dram_tensor("wk2", [C, D], f16, kind="ExternalInput")
    wv3x = nc.dram_tensor("wv3", [C, NH * D], f16, kind="ExternalInput")
    # per head, W_q columns duplicated [Qh|Qh] for the diag-packed S
    wqdx = nc.dram_tensor("wqd", [C, NH * P], f16, kind="ExternalInput")
    wpjx = nc.dram_tensor("wpj", [NH, D, C], f16, kind="ExternalInput")
    # bias cols: 0 K0|K1, 1 K2|-, 2 Q0|Q0, 3 Q1|Q1, 4 Q2|Q2
    biasx = nc.dram_tensor("bias", [P, 5], f32, kind="ExternalInput")
    bvx = nc.dram_tensor("bv16", [1, NH * D], f16, kind="ExternalInput")
    maskx = nc.dram_tensor("mask", [nslot, P, 1024], f16, kind="ExternalInput")
    out = nc.dram_tensor("out", [tq, C], f16, kind="ExternalOutput")

    with tile.TileContext(nc) as tc, ExitStack() as ctx:
        singles = ctx.enter_context(tc.tile_pool(name="singles", bufs=1))
        xthp = ctx.enter_context(tc.tile_pool(name="xth", bufs=2))
        xqp = ctx.enter_context(tc.tile_pool(name="xq", bufs=2))
        qtp = ctx.enter_context(tc.tile_pool(name="qt", bufs=2))
        mtp = ctx.enter_context(tc.tile_pool(name="mt", bufs=2))
        ptp = ctx.enter_context(tc.tile_pool(name="pt", bufs=4))
        rbp = ctx.enter_context(tc.tile_pool(name="rb", bufs=2))
        ytp = ctx.enter_context(tc.tile_pool(name="yt", bufs=2))
        ostp = ctx.enter_context(tc.tile_pool(name="ost", bufs=2))
        psg = ctx.enter_context(tc.tile_pool(name="psg", bufs=2, space="PSUM"))
        psy = ctx.enter_context(tc.tile_pool(name="psy", bufs=2, space="PSUM"))
        psm = ctx.enter_context(tc.tile_pool(name="psm", bufs=2, space="PSUM"))

        ones_f = singles.tile([1, 64], f32)
        nc.vector.memset(ones_f, 1.0)
        ones64 = singles.tile([1, 64], f32r)
        nc.vector.tensor_copy(ones64, ones_f)
        ones1h = singles.tile([1, P], f16)
        nc.vector.memset(ones1h, 1.0)
        # exp biases: softmax is shift-invariant.  -2 keeps off-diagonal
        # exp() inside fp8e4 range; the boundary group (holds the diagonal,
        # logit ~ |q|^2/8 up to ~13) uses -2-EB with V16 pre-scaled by e^EB.
        en2 = singles.tile([P, 1], f32)
        nc.vector.memset(en2, -FB)
        en6 = singles.tile([P, 1], f32)
        nc.vector.memset(en6, -BB)
        zbias = singles.tile([P, 1], f32)
        nc.vector.memset(zbias, 0.0)
        # dummy exp: pulls the ~2.7us ACT table load off the critical path
        scrp = singles.tile([1, 64], f32)
        nc.scalar.activation(scrp, ones_f, EXP, bias=zbias[0:1])

        wk01_t = singles.tile([P, 6, P], f16)
        wk2_t = singles.tile([P, 6, D], f16)
        wv3_t = singles.tile([P, 6, NH * D], f16)
        wqd_t = singles.tile([P, 6, NH * P], f16)
        for c in range(6):
            nc.sync.dma_start(wk01_t[:, c], wk01x[P * c:P * c + P, :])
            nc.sync.dma_start(wk2_t[:, c], wk2x[P * c:P * c + P, :])
            nc.sync.dma_start(wv3_t[:, c], wv3x[P * c:P * c + P, :])
            nc.sync.dma_start(wqd_t[:, c], wqdx[P * c:P * c + P, :])
        wpj_t = []
        for h in range(NH):
            w1 = singles.tile([D, C], f16, tag=f"wpj{h}", name=f"wpj{h}")
            nc.sync.dma_start(w1, wpjx[h])
            wpj_t.append(w1)
        b_t = singles.tile([P, 5], f32)
        nc.sync.dma_start(b_t, biasx[:, :])
        bv_t = singles.tile([1, NH * D], f16)
        nc.sync.dma_start(bv_t, bvx[:, :])

        # kt[h]: [128, T] diag-packed K^T.  Zero halves memset once (strided).
        kt_t = [singles.tile([P, Tloc], f16, tag=f"kt{h}", name=f"kt{h}")
                for h in range(NH)]
        for h in range(NH):
            v = kt_t[h].rearrange("p (b t) -> p b t", t=P)
            nc.vector.memset(v[0:64, :, 64:P], 0.0)
            nc.vector.memset(v[64:P, :, 0:64], 0.0)
        # vp8[h]: [128 k-part, pair, slab(2), 80]; col 64 = ones (denom).
        # vp16[h]: fp16 V^T copy scaled by e^EB for the boundary group.
        vp_t = [singles.tile([P, nkp, 2, 80], f8, tag=f"vp{h}", name=f"vp{h}")
                for h in range(NH)]
        vp16_t = [singles.tile([P, nkb, 65], f16, tag=f"vq{h}", name=f"vq{h}")
                  for h in range(NH)]
        for h in range(NH):
            nc.vector.memset(vp_t[h][:, :, :, 64:65], 1.0)
            nc.vector.memset(vp16_t[h][:, :, 64:65], float(np.exp(EB)))

        # ---- phase 1: K^T rows + V^T blocks for one column-quarter ----
        def p1_load(q):
            xh = xthp.tile([P, 6, QT4], f16, tag="xh", name=f"xh{q}")
            for c in range(6):
                nc.sync.dma_start(
                    xh[:, c], xt16[P * c:P * c + P, QT4 * q:QT4 * (q + 1)])
            return xh

        def kt_write(h, rows, g0, ps):
            """psum rows ([64] K^T dims x 512 t-cols) -> diag layout of kt[h]:
            even 64-col halves to top rows, odd halves to bottom rows."""
            pv = ps[rows].rearrange("p (b t) -> p b t", t=P)
            kv = kt_t[h][:, g0:g0 + 512].rearrange("p (b t) -> p b t", t=P)
            bsl = b_t[rows, 0:1] if h < 2 else b_t[0:64, 1:2]
            nc.vector.tensor_scalar(kv[0:64, :, 0:64], pv[:, :, 0:64],
                                    bsl, None, ADD)
            nc.vector.tensor_scalar(kv[64:P, :, 64:P], pv[:, :, 64:P],
                                    bsl, None, ADD)

        def p1_k01(q, xh, n0):
            g0 = QT4 * q + n0
            ps = psm.tile([P, 512], f32, tag="psm", name="psK")
            for c in range(6):
                nc.tensor.matmul(ps, wk01_t[:, c], xh[:, c, n0:n0 + 512],
                                 start=(c == 0), stop=(c == 5))
            kt_write(0, slice(0, 64), g0, ps)
            kt_write(1, slice(64, P), g0, ps)

        def p1_k2(q, xh, n0):
            g0 = QT4 * q + n0
            ps2 = psm.tile([P, 512], f32, tag="psm", name="psK2")
            for c in range(6):
                nc.tensor.matmul(ps2[0:64], wk2_t[:, c], xh[:, c, n0:n0 + 512],
                                 start=(c == 0), stop=(c == 5))
            kt_write(2, slice(0, 64), g0, ps2)

        def p1_v(q, xh, n0, bi):
            g0 = QT4 * q + n0
            t0 = n0 + P * bi
            blk = g0 // P + bi
            vps = psm.tile([P, 512], f32, tag="psm", name="psV")
            for c in range(6):
                nc.tensor.matmul(vps[:, :NH * D], xh[:, c, t0:t0 + P],
                                 wv3_t[:, c], start=(c == 0), stop=False)
            nc.tensor.matmul(vps[:, :NH * D], ones1h, bv_t,
                             start=False, stop=True)
            for h in range(NH):
                nc.vector.tensor_scalar(
                    vp16_t[h][:, blk, 0:64], vps[:, D * h:D * h + D],
                    float(np.exp(EB)), None, MUL)
                nc.vector.tensor_scalar(
                    vp_t[h][:, blk // 2, blk % 2, 0:64],
                    vp16_t[h][:, blk, 0:64],
                    float(np.exp(-EB)), None, MUL)

        def p1_pieces(q, xh, n0):
            yield lambda: p1_k01(q, xh, n0)
            yield lambda: p1_k2(q, xh, n0)
            for bi in range(4):
                yield lambda bi=bi: p1_v(q, xh, n0, bi)

        def p1_compute(q, xh, n0):
            for piece in p1_pieces(q, xh, n0):
                piece()

        # ---- Q^T (duplicated rows) for a slot pair (2 x 256 q rows) ----
        def q_proj(jp):
            xq = xqp.tile([P, 6, 512], f16, tag="xq", name="xq")
            for c in range(6):
                nc.sync.dma_start(
                    xq[:, c], xtq16[P * c:P * c + P, 512 * jp:512 * (jp + 1)])
            qts = []
            for h in range(NH):
                psq = psm.tile([P, 512], f32, tag="psm", name="psq")
                for c in range(6):
                    nc.tensor.matmul(psq, wqd_t[:, c, P * h:P * (h + 1)],
                                     xq[:, c], start=(c == 0), stop=(c == 5))
                qh = qtp.tile([P, 512], f16, tag=f"qt{h}", name=f"qt{h}")
                nc.vector.tensor_scalar(qh, psq, b_t[:, 2 + h:3 + h], None, ADD)
                qts.append(qh)
            return qts

        # ---- attention + projection for one slot ----
        def do_slot(j, qts, fillers=None):
            qoff = 256 * (j % 2)
            mt = mtp.tile([P, 1024], f16, tag="mt", name="mt")
            nc.sync.dma_start(mt, maskx[j])

            yt_t = []
            for h in range(NH):
                yacc = psy.tile([80, QC], f32, tag="yacc", name="yacc")
                qsl = qts[h][:, qoff:qoff + QC]

                def s_group(g):
                    sg = psg.tile([P, 1024], f32, tag="sg", name="sg")
                    for i in range(4):
                        kb = 4 * g + i
                        nc.tensor.matmul(sg[:, QC * i:QC * (i + 1)],
                                         kt_t[h][:, P * kb:P * (kb + 1)],
                                         qsl, start=True, stop=True)
                    return sg

                # software pipeline: issue S(g+1) before PV(g) so the PE has
                # work while ACT runs exp(g)
                sg_cur = s_group(0)
                for g in range(j + 1):
                    sg_next = s_group(g + 1) if g < j else None
                    if g < j:
                        # off-diagonal group: fp8 P + DoubleRow PV
                        pt = ptp.tile([P, 1024], f8, tag="pt", name="pt")
                        nc.scalar.activation(pt, sg_cur, EXP,
                                             bias=en2, scale=0.125)
                        for i2 in range(2):
                            nc.tensor.matmul(
                                yacc,
                                vp_t[h][:, 2 * g + i2],
                                pt[:, 512 * i2:512 * (i2 + 1)].rearrange(
                                    "p (s q) -> p s q", s=2),
                                start=(g == 0 and i2 == 0), stop=False,
                                perf_mode=DR)
                    else:
                        # boundary group holds the diagonal (logit ~ |q|^2/8,
                        # overflows fp8 and, at -2, even fp16): fp16 P with
                        # bias -2-EB against V16 pre-scaled by e^EB, masked
                        pt = ptp.tile([P, 1024], f16, tag="pt16", name="pt16")
                        nc.scalar.activation(pt, sg_cur, EXP,
                                             bias=en6, scale=0.125)
                        nc.vector.tensor_mul(pt, pt, mt)
                        for i in range(4):
                            kb = 4 * g + i
                            nc.tensor.matmul(yacc[0:65], vp16_t[h][:, kb],
                                             pt[:, QC * i:QC * (i + 1)],
                                             start=(g == 0 and i == 0),
                                             stop=(i == 3))
                    sg_cur = sg_next
                # y^T = yacc[0:64] / den; 1/den = exp(-ln(den)) on ACT over
                # the [1,256] ones-row (same act-table set as EXP), then a
                # K=1 ones matmul broadcasts it to 64 partitions
                lnd = rbp.tile([1, QC], f32, tag="lnd", name="lnd")
                nc.scalar.activation(lnd, yacc[64:65], LOG, bias=zbias[0:1])
                denr = rbp.tile([1, QC], f32r, tag="denr", name="denr")
                nc.scalar.activation(denr, lnd, EXP, bias=zbias[0:1],
                                     scale=-1.0)
                bc = psm.tile([P, 512], f32, tag="psm", name="bc")
                nc.tensor.matmul(bc[0:64, 0:QC], ones64, denr,
                                 start=True, stop=True)
                rb = rbp.tile([D, QC], f32, tag="rb", name="rb")
                nc.vector.tensor_copy(rb, bc[0:64, 0:QC])
                yt = ytp.tile([D, QC], f16, tag=f"yt{h}", name=f"yt{h}")
                nc.vector.tensor_mul(yt, yacc[0:64], rb)
                yt_t.append(yt)
                if fillers:
                    for _ in range(2):
                        if fillers:
                            fillers.pop(0)()

            ost = ostp.tile([P, 2, C], f16, tag="ost", name="ost")
            for qb in range(2):
                for (n0, nw) in [(0, 512), (512, 256)]:
                    pp = psm.tile([P, 512], f32, tag="psm", name="pp")
                    for h in range(NH):
                        nc.tensor.matmul(pp[:, :nw], yt_t[h][:, P * qb:P * (qb + 1)],
                                         wpj_t[h][:, n0:n0 + nw],
                                         start=(h == 0), stop=(h == NH - 1))
                    nc.vector.tensor_copy(ost[:, qb, n0:n0 + nw], pp[:, :nw])
            for qb in range(2):
                nc.sync.dma_start(
                    out[QC * j + P * qb:QC * j + P * (qb + 1), :], ost[:, qb])

        # ---- main schedule: phase-1 pieces drain as fillers between the
        # attention heads of the preceding slot pair (keeps PE dense) ----
        xh = p1_load(0)
        p1_compute(0, xh, 0)
        p1_compute(0, xh, 512)
        for jp in range(nslot // 2):
            fillers = []
            if jp + 1 < 4:
                xh = p1_load(jp + 1)
                fillers = list(p1_pieces(jp + 1, xh, 0)) + \
                    list(p1_pieces(jp + 1, xh, 512))
            qts = q_proj(jp)
            do_slot(2 * jp, qts, fillers)
            do_slot(2 * jp + 1, qts, fillers)
            for f in fillers:
                f()

    split_multi_waits(nc)
    return nc


def make_in_maps(x, W_qkv, b_qkv, W_proj, Tloc=T):
    """Shard the full inputs into the 8 per-core input maps."""
    nslot = Tloc // (2 * QC)
    xT = np.ascontiguousarray(x.reshape(Tloc, C).T).astype(np.float32)
    xT16 = xT.astype(np.float16)

    kk = np.arange(P)
    qq = np.arange(QC)
    in_maps = []
    for core in range(8):
        hg, s = core // 2, core % 2
        heads = [3 * hg + i for i in range(NH)]
        wk = [W_qkv[:, C + 64 * h:C + 64 * h + 64] for h in heads]
        wv = [W_qkv[:, 2 * C + 64 * h:2 * C + 64 * h + 64] for h in heads]
        wq = [W_qkv[:, 64 * h:64 * h + 64] for h in heads]
        wk01_c = np.concatenate(wk[0:2], axis=1).astype(np.float16)
        wk2_c = np.ascontiguousarray(wk[2]).astype(np.float16)
        wv3_c = np.concatenate(wv, axis=1).astype(np.float16)
        wqd_c = np.concatenate([np.tile(w, (1, 2)) for w in wq],
                               axis=1).astype(np.float16)
        wpj_c = np.stack([W_proj[64 * h:64 * h + 64, :] for h in heads]
                         ).astype(np.float16)

        bk = [b_qkv[C + 64 * h:C + 64 * h + 64] for h in heads]
        bv = [b_qkv[2 * C + 64 * h:2 * C + 64 * h + 64] for h in heads]
        bq = [b_qkv[64 * h:64 * h + 64] for h in heads]
        bias_c = np.zeros((P, 5), np.float32)
        bias_c[0:64, 0] = bk[0]
        bias_c[64:P, 0] = bk[1]
        bias_c[0:64, 1] = bk[2]
        for hi in range(NH):
            bias_c[0:64, 2 + hi] = bq[hi]
            bias_c[64:P, 2 + hi] = bq[hi]
        bv_c = np.concatenate(bv).reshape(1, NH * 64).astype(np.float16)

        qcols = np.concatenate(
            [np.arange(QC * (2 * j + s), QC * (2 * j + s) + QC)
             for j in range(nslot)])
        xtq_16 = np.ascontiguousarray(xT16[:, qcols])

        mask_c = np.zeros((nslot, P, 1024), np.float32)
        for j in range(nslot):
            q0 = QC * (2 * j + s)
            for i in range(4):
                k0 = P * (4 * j + i)
                mask_c[j, :, QC * i:QC * (i + 1)] = (
                    (k0 + kk[:, None]) <= (q0 + qq[None, :]))

        in_maps.append({
            "xt16": xT16, "xtq16": xtq_16,
            "wk01": wk01_c, "wk2": wk2_c, "wv3": wv3_c,
            "wqd": wqd_c, "wpj": wpj_c,
            "bias": bias_c, "bv16": bv_c,
            "mask": mask_c.astype(np.float16),
        })
    return in_maps


def unshard(results, b_proj, Tloc=T):
    nslot = Tloc // (2 * QC)
    out = np.zeros((Tloc, C), np.float64)
    for core in range(8):
        s = core % 2
        r = np.asarray(results[core]["out"]).astype(np.float64)
        for j in range(nslot):
            g0 = QC * (2 * j + s)
            out[g0:g0 + QC] += r[QC * j:QC * (j + 1)]
    out += b_proj.astype(np.float64)
    return out.astype(np.float32).reshape(1, Tloc, C)


_last_result = {}


def kernel(x, mask, W_qkv, b_qkv, W_proj, b_proj):
    from concourse.bass_utils import run_bass_kernel_spmd
    x = np.asarray(x, np.float32)
    W_qkv = np.asarray(W_qkv, np.float32)
    b_qkv = np.asarray(b_qkv, np.float32)
    W_proj = np.asarray(W_proj, np.float32)
    b_proj = np.asarray(b_proj, np.float32)

    if "nc" not in _nc_cache:
        _nc_cache["nc"] = build_nc(T)
    nc = _nc_cache["nc"]
    in_maps = make_in_maps(x, W_qkv, b_qkv, W_proj, T)
    import os
    kwargs = {}
    if os.environ.get("BASS_KERNEL_TRACE"):
        kwargs = dict(trace=True, trace_cores=list(range(8)))
    res = run_bass_kernel_spmd(nc, in_maps, core_ids=list(range(8)), **kwargs)
    _last_result["res"] = res
    return unshard([r for r in res.results], b_proj, T)
